# revision 20
# baseline (speedup 1.0000x reference)
"""Trainium2 Bass kernel for nn_Attention2 (single-head attention, row-0 output).

Reference computes full attention out = softmax(q k^T / sqrt(d)) v per (b, inst)
pair and returns only out[:, :, 0, :].  Only query row 0 therefore matters:

    c   = x0 @ (Wq^T Wk) / sqrt(d)        # x0 = x[b,i,0,:], M1 precomputed on host
    s_l = sum_d c_d x[l,d]                # scores, |s| ~ 2 -> exp safe w/o max-sub
    e   = exp(s);  z = sum_l e_l
    out = (e @ x) @ Wv^T / z

Per (b, inst) pair that is two 512-wide matvecs over x instead of five 512^3
matmuls.  The kernel streams x (32 MB per core) through SBUF once; the score
matvec (contracts d, the free axis of the natural x layout) runs on the vector
engine as fused multiply+reduce, the value matvec (contracts l, the partition
axis) and all dense matmuls run on the tensor engine in fp32r.

8 cores are pure data-parallel over the 256 (b, inst) pairs (32 pairs each).
"""

import numpy as np

import concourse.bass as bass
import concourse.tile as tile
from concourse import bacc, bass_utils, mybir
from concourse.bass import ts

F32 = mybir.dt.float32
R32 = mybir.dt.float32r

N_CORES = 8
B, INST, L, D = 8, 32, 512, 512
P = 128
LT = L // P  # 4 l-tiles
DT = D // P  # 4 d-tiles
PAIRS = (B * INST) // N_CORES  # 32 pairs per core


def _r(ap):
    return ap.bitcast(R32)


def _build_program(pairs=PAIRS, reps=1, mode="full"):
    nc = bacc.Bacc(
        "TRN2",
        target_bir_lowering=False,
        debug=False,
        num_devices=N_CORES,
    )

    selp = LT * pairs  # rows of the lt-collapse selection matrix

    x_t = nc.dram_tensor("x", [pairs, L, D], F32, kind="ExternalInput")
    m1_t = nc.dram_tensor("m1", [D, D], F32, kind="ExternalInput")
    wvt_t = nc.dram_tensor("wvt", [D, D], F32, kind="ExternalInput")
    ident_t = nc.dram_tensor("ident", [P, P], F32, kind="ExternalInput")
    sel_t = nc.dram_tensor("sel", [selp, pairs], F32, kind="ExternalInput")
    ones_col_t = nc.dram_tensor("ones_col", [P, 1], F32, kind="ExternalInput")
    ones_row_t = nc.dram_tensor("ones_row", [1, P], F32, kind="ExternalInput")
    out_t = nc.dram_tensor("out", [pairs, D], F32, kind="ExternalOutput")

    x_ap = x_t.ap()

    with tile.TileContext(nc) as tc:
        with (
            tc.tile_pool(name="consts", bufs=1) as consts,
            tc.tile_pool(name="x", bufs=9) as xpool,
            tc.tile_pool(name="xr", bufs=9) as xrpool,
            tc.tile_pool(name="work", bufs=4) as sbuf,
            tc.tile_pool(name="accum", bufs=1) as accum,
            tc.tile_pool(name="psum", bufs=1, space="PSUM") as psum,
            tc.tile_pool(name="dram", bufs=1, space="DRAM") as dram,
            tc.tile_pool(name="psum2", bufs=4, space="PSUM") as psum2,
            tc.tile_pool(name="psum3", bufs=2, space="PSUM") as psum3,
        ):
            m1_sb = consts.tile([P, DT, D], F32)
            nc.sync.dma_start(m1_sb, m1_t.ap().rearrange("(dt p) d -> p dt d", p=P))
            wvt_sb = consts.tile([P, DT, D], F32)
            nc.sync.dma_start(wvt_sb, wvt_t.ap().rearrange("(dt p) d -> p dt d", p=P))
            ident_sb = consts.tile([P, P], F32)
            nc.sync.dma_start(ident_sb, ident_t.ap())
            sel_sb = consts.tile([selp, pairs], F32)
            nc.sync.dma_start(sel_sb, sel_t.ap())
            ones_col = consts.tile([P, 1], F32)
            nc.sync.dma_start(ones_col, ones_col_t.ap())
            ones_row = consts.tile([1, P], F32)
            nc.sync.dma_start(ones_row, ones_row_t.ap())

            # ---- phase A: x0 rows for all pairs, transposed to [d, pair] ----
            x0_rows = consts.tile([pairs, D], F32)
            nc.sync.dma_start(x0_rows, x_ap[:, 0, :])
            x0t_ps = psum.tile([P, DT * pairs], F32, tag="x0t")
            for dt in range(DT):
                nc.tensor.transpose(
                    x0t_ps[:, ts(dt, pairs)],
                    x0_rows[:, ts(dt, P)],
                    ident_sb[:pairs, :pairs],
                )
            x0t_sb = consts.tile([P, DT, pairs], R32)
            nc.scalar.copy(x0t_sb.rearrange("p dt j -> p (dt j)"), x0t_ps[:])
            m1_r = consts.tile([P, DT, D], R32)
            nc.scalar.copy(m1_r.rearrange("p dt d -> p (dt d)"),
                           m1_sb.rearrange("p dt d -> p (dt d)"))
            ones_row_r = consts.tile([1, P], R32)
            nc.scalar.copy(ones_row_r, ones_row[:])
            ones_col2_r = consts.tile([P, 2], R32)
            nc.scalar.copy(ones_col2_r, ones_col.to_broadcast((P, 2)))

            # batched C = X0 @ M1 for all pairs (fp32r, one accumulation group)
            c_ps = psum.tile([pairs, D], F32, tag="x0t")
            for dt in range(DT):
                nc.tensor.matmul(
                    c_ps[:],
                    x0t_sb[:, dt, :],
                    m1_r[:, dt, :],
                    start=(dt == 0),
                    stop=(dt == DT - 1),
                )
            c_all_sb = consts.tile([pairs, D], F32)
            nc.scalar.copy(c_all_sb, c_ps[:])
            c_dram = dram.tile([pairs, D], F32)
            nc.sync.dma_start(c_dram, c_all_sb)

            # e_all[p, j, lt] = exp(s[j, lt*128+p]); uet_all[d%128, dt, j] = ue[j, d]/..
            e_all = accum.tile([P, pairs, LT], R32)
            uet_all = accum.tile([P, DT, pairs], F32)

            scratch = accum.tile([P, D], F32)  # discarded ttr elementwise output

            if mode == "noue":
                nc.gpsimd.memset(uet_all.rearrange("p a b -> p (a b)"), 0.01)

            for j in [jj % pairs for jj in range(reps * pairs)]:
                x_sb = xpool.tile([P, LT, D], F32, tag="x")
                nc.sync.dma_start(x_sb, x_ap[j].rearrange("(lt p) d -> p lt d", p=P))
                if mode == "dmaonly":
                    continue
                x_r = xrpool.tile([P, LT, D], R32, tag="xr")
                nc.scalar.copy(
                    x_r[:, 0:3, :].rearrange("p lt d -> p (lt d)"),
                    x_sb[:, 0:3, :].rearrange("p lt d -> p (lt d)"),
                )
                nc.vector.tensor_copy(
                    x_r[:, 3:4, :].rearrange("p lt d -> p (lt d)"),
                    x_sb[:, 3:4, :].rearrange("p lt d -> p (lt d)"),
                )

                # broadcast C row j across 128 partitions (SBUF->SBUF DMA)
                cb_sb = sbuf.tile([P, D], F32, tag="cb_sb")
                nc.sync.dma_start(
                    cb_sb, c_dram[j : j + 1, :].partition_broadcast(P)
                )

                # scores: s[p, lt] = sum_d x[p, lt, d] * c[d]   (DVE fused mul+reduce)
                s_col = sbuf.tile([P, LT], F32, tag="s")
                if mode == "noscore":
                    nc.vector.memset(s_col, 0.5)
                for lt in range(LT) if mode != "noscore" else []:
                    nc.vector.affine_mul_reduce(
                        out=scratch[:],
                        accum_out=s_col[:, lt : lt + 1],
                        in0=x_sb[:, lt, :],
                        in1=cb_sb[:],
                        scale=1.0,
                        bias=0.0,
                    )

                nc.scalar.activation(
                    e_all[:, j, :], s_col[:], mybir.ActivationFunctionType.Exp
                )

                if mode == "noue":
                    continue
                # ue[j] = e[j] @ x[j]  -> [1, D] on PE (fp32r, N=512)
                ue_ps = psum2.tile([1, D], F32, tag="ue")
                for lt in range(LT):
                    nc.tensor.matmul(
                        ue_ps[:],
                        e_all[:, j, lt : lt + 1],
                        x_r[:, lt, :],
                        start=(lt == 0),
                        stop=(lt == LT - 1),
                    )
                ue_sb = sbuf.tile([1, D], F32, tag="ue_sb")
                nc.scalar.copy(ue_sb, ue_ps[:])

                # transpose ue row -> uet_all[:, dt, j] (tiny N=1 matmuls)
                uet_ps = psum3.tile([P, DT], F32, tag="uet")
                for dt in range(DT):
                    nc.tensor.matmul(
                        uet_ps[:, dt : dt + 1],
                        ue_sb[0:1, ts(dt, P)],
                        ones_row[0:1, 0:1],
                    )
                nc.scalar.copy(uet_all[:, :, j], uet_ps[:])

            if mode != "dmaonly":
                _tail(nc, tc, pairs, selp, sbuf, psum, psum2, psum3, e_all, uet_all,
                      ones_col2_r, sel_sb, wvt_sb, out_t)

    nc.compile()
    return nc


def _tail(nc, tc, pairs, selp, sbuf, psum, psum2, psum3, e_all, uet_all,
          ones_col2_r, sel_sb, wvt_sb, out_t):
            # ---- normalization: z[j] = sum over (p, lt) of e_all ----
            zpart_ps = psum.tile([selp, 2], F32, tag="x0t")
            nc.tensor.matmul(
                zpart_ps[:],
                e_all.rearrange("p j lt -> p (j lt)"),
                ones_col2_r[:],
            )
            zpart_sb = sbuf.tile([selp, 1], F32, tag="zpart")
            nc.scalar.copy(zpart_sb, zpart_ps[:, 0:1])
            zcol_ps = psum3.tile([pairs, 1], F32, tag="uet")
            nc.tensor.matmul(zcol_ps[:], sel_sb[:], zpart_sb[:])
            zcol_sb = sbuf.tile([pairs, 1], F32, tag="zcol")
            nc.scalar.copy(zcol_sb, zcol_ps[:])
            zi_sb = sbuf.tile([pairs, 1], F32, tag="zi")
            nc.vector.reciprocal(zi_sb, zcol_sb)

            # ---- out = (uet^T @ WvT) * zi ----
            out_ps = psum2.tile([pairs, D], F32, tag="ue")
            for dt in range(DT):
                nc.tensor.matmul(
                    out_ps[:],
                    uet_all[:, dt, :],
                    wvt_sb[:, dt, :],
                    start=(dt == 0),
                    stop=(dt == DT - 1),
                )
            out_sb = sbuf.tile([pairs, D], F32, tag="out")
            nc.scalar.activation(
                out_sb,
                out_ps[:],
                mybir.ActivationFunctionType.Copy,
                scale=zi_sb[:],
            )
            nc.sync.dma_start(out_t.ap(), out_sb)


def _host_consts(pairs=PAIRS):
    ident = np.eye(P, dtype=np.float32)
    sel = np.zeros((LT * pairs, pairs), dtype=np.float32)
    for m in range(LT * pairs):
        sel[m, m // LT] = 1.0
    ones_col = np.ones((P, 1), dtype=np.float32)
    ones_row = np.ones((1, P), dtype=np.float32)
    return ident, sel, ones_col, ones_row


_NC_CACHE = {}


def kernel(x, Wq, Wk, Wv):
    x = np.ascontiguousarray(np.asarray(x, dtype=np.float32))
    Wq = np.asarray(Wq, dtype=np.float32)
    Wk = np.asarray(Wk, dtype=np.float32)
    Wv = np.asarray(Wv, dtype=np.float32)

    temp = np.sqrt(np.float32(D)).astype(np.float64)
    m1 = ((Wq.T.astype(np.float64) @ Wk.astype(np.float64)) / temp).astype(np.float32)
    wvt = np.ascontiguousarray(Wv.T)

    if "nc" not in _NC_CACHE:
        _NC_CACHE["nc"] = _build_program()
    nc = _NC_CACHE["nc"]

    ident, sel, ones_col, ones_row = _host_consts()
    shards = x.reshape(N_CORES, PAIRS, L, D)
    in_maps = [
        {
            "x": shards[c],
            "m1": m1,
            "wvt": wvt,
            "ident": ident,
            "sel": sel,
            "ones_col": ones_col,
            "ones_row": ones_row,
        }
        for c in range(N_CORES)
    ]
    res = bass_utils.run_bass_kernel_spmd(
        nc, in_maps, core_ids=list(range(N_CORES)), trace=False
    )
    out = np.stack([res.results[c]["out"] for c in range(N_CORES)])
    return out.reshape(B, INST, D)


# revision 21
# speedup vs baseline: 225065.7471x; 225065.7471x over previous
"""Trainium2 Bass kernel for nn_Attention2 (single-head attention, row-0 output).

Reference computes full attention out = softmax(q k^T / sqrt(d)) v per (b, inst)
pair and returns only out[:, :, 0, :].  Only query row 0 therefore matters:

    c   = x0 @ (Wq^T Wk) / sqrt(d)        # x0 = x[b,i,0,:], M1 precomputed on host
    s_l = sum_d c_d x[l,d]                # scores, |s| ~ 2 -> exp safe w/o max-sub
    e   = exp(s);  z = sum_l e_l
    out = (e @ x) @ Wv^T / z

Per (b, inst) pair that is two 512-wide matvecs over x instead of five 512^3
matmuls.  The kernel streams x (32 MB per core) through SBUF once; the score
matvec (contracts d, the free axis of the natural x layout) runs on the vector
engine as fused multiply+reduce, the value matvec (contracts l, the partition
axis) and all dense matmuls run on the tensor engine in fp32r.

8 cores are pure data-parallel over the 256 (b, inst) pairs (32 pairs each).
"""

import numpy as np

import concourse.tile as tile
from concourse import bacc, bass_utils, mybir
from concourse.bass import ts

F32 = mybir.dt.float32
R32 = mybir.dt.float32r

N_CORES = 8
B, INST, L, D = 8, 32, 512, 512
P = 128
LT = L // P  # 4 l-tiles
DT = D // P  # 4 d-tiles
PAIRS = (B * INST) // N_CORES  # 32 pairs per core


def _build_program(pairs=PAIRS, reps=1, mode="full"):
    nc = bacc.Bacc(
        "TRN2",
        target_bir_lowering=False,
        debug=False,
        num_devices=N_CORES,
    )

    selp = LT * pairs  # rows of the lt-collapse selection matrix

    x_t = nc.dram_tensor("x", [pairs, L, D], F32, kind="ExternalInput")
    m1_t = nc.dram_tensor("m1", [D, D], F32, kind="ExternalInput")
    wvt_t = nc.dram_tensor("wvt", [D, D], F32, kind="ExternalInput")
    ident_t = nc.dram_tensor("ident", [P, P], F32, kind="ExternalInput")
    sel_t = nc.dram_tensor("sel", [selp, pairs], F32, kind="ExternalInput")
    ones_col_t = nc.dram_tensor("ones_col", [P, 1], F32, kind="ExternalInput")
    ones_row_t = nc.dram_tensor("ones_row", [1, P], F32, kind="ExternalInput")
    out_t = nc.dram_tensor("out", [pairs, D], F32, kind="ExternalOutput")

    x_ap = x_t.ap()

    with tile.TileContext(nc) as tc:
        with (
            tc.tile_pool(name="consts", bufs=1) as consts,
            tc.tile_pool(name="x", bufs=9) as xpool,
            tc.tile_pool(name="xr", bufs=9) as xrpool,
            tc.tile_pool(name="work", bufs=4) as sbuf,
            tc.tile_pool(name="accum", bufs=1) as accum,
            tc.tile_pool(name="psum", bufs=1, space="PSUM") as psum,
            tc.tile_pool(name="dram", bufs=1, space="DRAM") as dram,
            tc.tile_pool(name="psum2", bufs=4, space="PSUM") as psum2,
            tc.tile_pool(name="psum3", bufs=2, space="PSUM") as psum3,
        ):
            m1_sb = consts.tile([P, DT, D], F32)
            nc.sync.dma_start(m1_sb, m1_t.ap().rearrange("(dt p) d -> p dt d", p=P))
            wvt_sb = consts.tile([P, DT, D], F32)
            nc.sync.dma_start(wvt_sb, wvt_t.ap().rearrange("(dt p) d -> p dt d", p=P))
            ident_sb = consts.tile([P, P], F32)
            nc.sync.dma_start(ident_sb, ident_t.ap())
            sel_sb = consts.tile([selp, pairs], F32)
            nc.sync.dma_start(sel_sb, sel_t.ap())
            ones_col = consts.tile([P, 1], F32)
            nc.sync.dma_start(ones_col, ones_col_t.ap())
            ones_row = consts.tile([1, P], F32)
            nc.sync.dma_start(ones_row, ones_row_t.ap())

            # ---- phase A: x0 rows for all pairs, transposed to [d, pair] ----
            x0_rows = consts.tile([pairs, D], F32)
            nc.sync.dma_start(x0_rows, x_ap[:, 0, :])
            x0t_ps = psum.tile([P, DT * pairs], F32, tag="x0t")
            for dt in range(DT):
                nc.tensor.transpose(
                    x0t_ps[:, ts(dt, pairs)],
                    x0_rows[:, ts(dt, P)],
                    ident_sb[:pairs, :pairs],
                )
            x0t_sb = consts.tile([P, DT, pairs], R32)
            nc.scalar.copy(x0t_sb.rearrange("p dt j -> p (dt j)"), x0t_ps[:])
            m1_r = consts.tile([P, DT, D], R32)
            nc.scalar.copy(m1_r.rearrange("p dt d -> p (dt d)"),
                           m1_sb.rearrange("p dt d -> p (dt d)"))
            ones_col2_r = consts.tile([P, 2], R32)
            nc.scalar.copy(ones_col2_r, ones_col.to_broadcast((P, 2)))

            # batched C = X0 @ M1 for all pairs (fp32r, one accumulation group)
            c_ps = psum.tile([pairs, D], F32, tag="x0t")
            for dt in range(DT):
                nc.tensor.matmul(
                    c_ps[:],
                    x0t_sb[:, dt, :],
                    m1_r[:, dt, :],
                    start=(dt == 0),
                    stop=(dt == DT - 1),
                )
            c_all_sb = consts.tile([pairs, D], F32)
            nc.scalar.copy(c_all_sb, c_ps[:])
            c_dram = dram.tile([pairs, D], F32)
            nc.sync.dma_start(c_dram, c_all_sb)

            # e_all[p, j, lt] = exp(s[j, lt*128+p]); uet_all[d%128, dt, j] = ue[j, d]/..
            e_all = accum.tile([P, pairs, LT], R32)
            uet_all = accum.tile([P, DT, pairs], F32)

            scratch = accum.tile([P, D], F32)  # discarded ttr elementwise output

            if mode == "noue":
                nc.gpsimd.memset(uet_all.rearrange("p a b -> p (a b)"), 0.01)

            for j in [jj % pairs for jj in range(reps * pairs)]:
                x_sb = xpool.tile([P, LT, D], F32, tag="x")
                nc.sync.dma_start(x_sb, x_ap[j].rearrange("(lt p) d -> p lt d", p=P))
                if mode == "dmaonly":
                    continue
                x_r = xrpool.tile([P, LT, D], R32, tag="xr")
                nc.scalar.copy(
                    x_r[:, 0:3, :].rearrange("p lt d -> p (lt d)"),
                    x_sb[:, 0:3, :].rearrange("p lt d -> p (lt d)"),
                )
                nc.vector.tensor_copy(
                    x_r[:, 3:4, :].rearrange("p lt d -> p (lt d)"),
                    x_sb[:, 3:4, :].rearrange("p lt d -> p (lt d)"),
                )

                # broadcast C row j across 128 partitions (SBUF->SBUF DMA)
                cb_sb = sbuf.tile([P, D], F32, tag="cb_sb")
                nc.sync.dma_start(
                    cb_sb, c_dram[j : j + 1, :].partition_broadcast(P)
                )

                # scores: s[p, lt] = sum_d x[p, lt, d] * c[d]   (DVE fused mul+reduce)
                s_col = sbuf.tile([P, LT], F32, tag="s")
                if mode == "noscore":
                    nc.vector.memset(s_col, 0.5)
                for lt in range(LT) if mode != "noscore" else []:
                    nc.vector.affine_mul_reduce(
                        out=scratch[:],
                        accum_out=s_col[:, lt : lt + 1],
                        in0=x_sb[:, lt, :],
                        in1=cb_sb[:],
                        scale=1.0,
                        bias=0.0,
                    )

                nc.scalar.activation(
                    e_all[:, j, :], s_col[:], mybir.ActivationFunctionType.Exp
                )

                if mode == "noue":
                    continue
                # ue[j] = e[j] @ x[j]  -> [1, D] on PE (fp32r, N=512)
                ue_ps = psum2.tile([1, D], F32, tag="ue")
                for lt in range(LT):
                    nc.tensor.matmul(
                        ue_ps[:],
                        e_all[:, j, lt : lt + 1],
                        x_r[:, lt, :],
                        start=(lt == 0),
                        stop=(lt == LT - 1),
                    )
                ue_sb = sbuf.tile([1, D], F32, tag="ue_sb")
                nc.scalar.copy(ue_sb, ue_ps[:])

                # transpose ue row -> uet_all[:, dt, j] (tiny N=1 matmuls)
                uet_ps = psum3.tile([P, DT], F32, tag="uet")
                for dt in range(DT):
                    nc.tensor.matmul(
                        uet_ps[:, dt : dt + 1],
                        ue_sb[0:1, ts(dt, P)],
                        ones_row[0:1, 0:1],
                    )
                nc.scalar.copy(uet_all[:, :, j], uet_ps[:])

            if mode != "dmaonly":
                _tail(nc, tc, pairs, selp, sbuf, psum, psum2, psum3, e_all, uet_all,
                      ones_col2_r, sel_sb, wvt_sb, out_t)

    nc.compile()
    return nc


def _tail(nc, tc, pairs, selp, sbuf, psum, psum2, psum3, e_all, uet_all,
          ones_col2_r, sel_sb, wvt_sb, out_t):
            # ---- normalization: z[j] = sum over (p, lt) of e_all ----
            zpart_ps = psum.tile([selp, 2], F32, tag="x0t")
            nc.tensor.matmul(
                zpart_ps[:],
                e_all.rearrange("p j lt -> p (j lt)"),
                ones_col2_r[:],
            )
            zpart_sb = sbuf.tile([selp, 1], F32, tag="zpart")
            nc.scalar.copy(zpart_sb, zpart_ps[:, 0:1])
            zcol_ps = psum3.tile([pairs, 1], F32, tag="uet")
            nc.tensor.matmul(zcol_ps[:], sel_sb[:], zpart_sb[:])
            zcol_sb = sbuf.tile([pairs, 1], F32, tag="zcol")
            nc.scalar.copy(zcol_sb, zcol_ps[:])
            zi_sb = sbuf.tile([pairs, 1], F32, tag="zi")
            nc.vector.reciprocal(zi_sb, zcol_sb)

            # ---- out = (uet^T @ WvT) * zi ----
            out_ps = psum2.tile([pairs, D], F32, tag="ue")
            for dt in range(DT):
                nc.tensor.matmul(
                    out_ps[:],
                    uet_all[:, dt, :],
                    wvt_sb[:, dt, :],
                    start=(dt == 0),
                    stop=(dt == DT - 1),
                )
            out_sb = sbuf.tile([pairs, D], F32, tag="out")
            nc.scalar.activation(
                out_sb,
                out_ps[:],
                mybir.ActivationFunctionType.Copy,
                scale=zi_sb[:],
            )
            nc.sync.dma_start(out_t.ap(), out_sb)


def _host_consts(pairs=PAIRS):
    ident = np.eye(P, dtype=np.float32)
    sel = np.zeros((LT * pairs, pairs), dtype=np.float32)
    for m in range(LT * pairs):
        sel[m, m // LT] = 1.0
    ones_col = np.ones((P, 1), dtype=np.float32)
    ones_row = np.ones((1, P), dtype=np.float32)
    return ident, sel, ones_col, ones_row


_NC_CACHE = {}


def kernel(x, Wq, Wk, Wv):
    x = np.ascontiguousarray(np.asarray(x, dtype=np.float32))
    Wq = np.asarray(Wq, dtype=np.float32)
    Wk = np.asarray(Wk, dtype=np.float32)
    Wv = np.asarray(Wv, dtype=np.float32)

    temp = np.sqrt(np.float32(D)).astype(np.float64)
    m1 = ((Wq.T.astype(np.float64) @ Wk.astype(np.float64)) / temp).astype(np.float32)
    wvt = np.ascontiguousarray(Wv.T)

    if "nc" not in _NC_CACHE:
        _NC_CACHE["nc"] = _build_program()
    nc = _NC_CACHE["nc"]

    ident, sel, ones_col, ones_row = _host_consts()
    shards = x.reshape(N_CORES, PAIRS, L, D)
    in_maps = [
        {
            "x": shards[c],
            "m1": m1,
            "wvt": wvt,
            "ident": ident,
            "sel": sel,
            "ones_col": ones_col,
            "ones_row": ones_row,
        }
        for c in range(N_CORES)
    ]
    res = bass_utils.run_bass_kernel_spmd(
        nc, in_maps, core_ids=list(range(N_CORES)), trace=False
    )
    out = np.stack([res.results[c]["out"] for c in range(N_CORES)])
    return out.reshape(B, INST, D)
